# revision 9
# baseline (speedup 1.0000x reference)
"""DenseCL head loss kernel for Trainium2 (8 NeuronCores, batch-parallel).

Per-core shard: 8 of the 64 samples. On-device per sample:
  pred = W2 @ relu(W1 @ dense_on + b1) + b2            (MLP over channels)
  G    = feat_on^T @ feat_targ  (per-position gram)     -> argmax_j G[:,j]/|ft_j|
  P    = pred^T @ [dense_targ | pred]                   (dot + pred-norm diag)
  cos  = P[i, idx_i] / sqrt(|pred_i|^2 * |dt_idx_i|^2)
Core output = sum_i cos (scalar partial). Host combines partials:
  loss = -2 * S / (b*h*w) + 2

All matmuls run in bf16 with fp32 PSUM accumulation; the scalar tail
(norms, argmax compare, final cosine) is fp32. Inputs are cast to bf16 and
laid out in SBUF order (partition-major) on the host, so every device DMA
is a large fully-contiguous transfer.
"""

import numpy as np
import ml_dtypes

import concourse.bacc as bacc
import concourse.bass as bass
import concourse.mybir as mybir
import concourse.tile as tile

F32 = mybir.dt.float32
BF16 = mybir.dt.bfloat16
U32 = mybir.dt.uint32
AF = mybir.ActivationFunctionType
ALU = mybir.AluOpType

# problem shapes (hardcoded per spec)
B_FULL, CF, H, W = 64, 2048, 14, 14
CD, HID = 256, 2048
HW = H * W                       # 196
N_CORES = 8
BSH = B_FULL // N_CORES          # 8 samples per core
KF = CF // 128                   # 16 feat k-tiles
KD = CD // 128                   # 2 dense k-tiles
KH = HID // 128                  # 16 hidden k-tiles
MT = [(0, 128), (128, HW - 128)]  # m-tiles over the 196 positions
NPAIR = 2 * HW                   # 392: two samples side by side
FHALF = KF // 2                  # feat DMA split for pipelining


def build_nc():
    nc = bacc.Bacc("TRN2", target_bir_lowering=False, debug=False,
                   num_devices=N_CORES)

    # host pre-arranged, bf16, partition-major
    f_on = nc.dram_tensor("f_on", [BSH, 128, KF, HW], BF16, kind="ExternalInput")
    f_tg = nc.dram_tensor("f_tg", [BSH, 128, KF, HW], BF16, kind="ExternalInput")
    d_on = nc.dram_tensor("d_on", [128, KD, BSH, HW], BF16, kind="ExternalInput")
    d_tg = nc.dram_tensor("d_tg", [128, KD, BSH, HW], BF16, kind="ExternalInput")
    w1t = nc.dram_tensor("w1t", [128, KD, HID], BF16, kind="ExternalInput")
    w2t = nc.dram_tensor("w2t", [128, KH, CD], BF16, kind="ExternalInput")
    b1r = nc.dram_tensor("b1r", [128, KH], F32, kind="ExternalInput")
    b2r = nc.dram_tensor("b2r", [128, KD], F32, kind="ExternalInput")
    out = nc.dram_tensor("out", [1, 1], F32, kind="ExternalOutput")

    with tile.TileContext(nc) as tc:
        with (
            tc.tile_pool(name="singles", bufs=1) as singles,
            tc.tile_pool(name="fpool", bufs=4) as fpool,
            tc.tile_pool(name="sqpool", bufs=2) as sqpool,
            tc.tile_pool(name="hpool", bufs=3) as hpool,
            tc.tile_pool(name="cospool", bufs=3) as cospool,
            tc.tile_pool(name="smalls", bufs=3) as smalls,
            tc.tile_pool(name="idxpool", bufs=6) as idxpool,
            tc.tile_pool(name="ps_h", bufs=2, space="PSUM") as ps_h,
            tc.tile_pool(name="ps_pred", bufs=2, space="PSUM") as ps_pred,
            tc.tile_pool(name="ps_big", bufs=2, space="PSUM") as ps_big,
            tc.tile_pool(name="ps_small", bufs=2, space="PSUM") as ps_small,
        ):
            # ---- MLP inputs first: PE can start on the MLP while feats load
            w1sb = singles.tile([128, KD, HID], BF16)
            nc.sync.dma_start(out=w1sb, in_=w1t.ap())
            xsb = singles.tile([128, KD, BSH, HW], BF16)
            nc.sync.dma_start(out=xsb, in_=d_on.ap())
            w2sb = singles.tile([128, KH, CD], BF16)
            nc.sync.dma_start(out=w2sb, in_=w2t.ap())
            b1sb = singles.tile([128, KH], F32)
            nc.sync.dma_start(out=b1sb, in_=b1r.ap())
            b2sb = singles.tile([128, KD], F32)
            nc.sync.dma_start(out=b2sb, in_=b2r.ap())

            f1sb = {}
            f2sb = {}

            def load_feats(b):
                f1 = fpool.tile([128, KF * HW], BF16, tag="f1")
                f2 = fpool.tile([128, KF * HW], BF16, tag="f2")
                half = FHALF * HW
                for h0 in (0, 1):
                    nc.sync.dma_start(
                        out=f2[:, h0 * half:(h0 + 1) * half],
                        in_=f_tg.ap()[b, :, h0 * FHALF:(h0 + 1) * FHALF])
                    nc.sync.dma_start(
                        out=f1[:, h0 * half:(h0 + 1) * half],
                        in_=f_on.ap()[b, :, h0 * FHALF:(h0 + 1) * FHALF])
                f1sb[b] = f1
                f2sb[b] = f2

            load_feats(0)
            load_feats(1)

            # C holds [dense_targ | pred] per (k-tile, sample): width 392
            csb = singles.tile([128, KD, BSH, 2 * HW], BF16)
            for k in range(KD):
                nc.sync.dma_start(out=csb[:, k, :, :HW], in_=d_tg.ap()[:, k])

            ones_b = singles.tile([128, 1], BF16)
            nc.vector.memset(ones_b, 1.0)
            ones_f = singles.tile([128, 1], F32)
            nc.vector.memset(ones_f, 1.0)
            iota_j = singles.tile([128, HW], F32)
            nc.gpsimd.iota(iota_j, [[1, HW]], channel_multiplier=0,
                           allow_small_or_imprecise_dtypes=True)
            iota_d = singles.tile([128, HW], F32)  # value = n - p
            nc.gpsimd.iota(iota_d, [[1, HW]], channel_multiplier=-1,
                           allow_small_or_imprecise_dtypes=True)
            # result accumulator: res[p, m*BSH + b] = cos for position m*128+p
            res = singles.tile([128, 2 * BSH], F32)
            nc.vector.memset(res, 0.0)

            idxf = {}

            def stage_a(b):
                """gram + argmax for sample b -> idxf[b] (per-mtile (mw,1))."""
                f1, f2 = f1sb[b], f2sb[b]
                # |ft_j|^2 via ones-matmul over squared f2 (two halves so the
                # squares pipeline with the second half of the f2 DMA)
                f2sq = sqpool.tile([128, KF * HW], BF16, tag="f2sq")
                half = FHALF * HW
                for h0 in (0, 1):
                    nc.vector.tensor_mul(
                        f2sq[:, h0 * half:(h0 + 1) * half],
                        f2[:, h0 * half:(h0 + 1) * half],
                        f2[:, h0 * half:(h0 + 1) * half])
                nrm_ps = ps_small.tile([1, HW], F32, tag="small")
                for k in range(KF):
                    nc.tensor.matmul(
                        nrm_ps, ones_b, f2sq[:, k * HW:(k + 1) * HW],
                        start=(k == 0), stop=(k == KF - 1))
                # 1/|ft_j|
                rn = smalls.tile([1, HW], F32, tag="rn")
                nc.scalar.sqrt(out=rn, in_=nrm_ps)
                nc.vector.reciprocal_approx_fast(out=rn, in_=rn)
                rnb = smalls.tile([128, HW], F32, tag="rnb")
                nc.gpsimd.partition_broadcast(rnb, rn)

                idxf[b] = []
                for mi, (m0, mw) in enumerate(MT):
                    g_ps = ps_big.tile([128, HW], F32, tag="big")
                    for k in range(KF):
                        nc.tensor.matmul(
                            g_ps[:mw],
                            f1[:, k * HW + m0: k * HW + m0 + mw],
                            f2[:, k * HW:(k + 1) * HW],
                            start=(k == 0), stop=(k == KF - 1))
                    cosm = cospool.tile([128, HW], F32, tag="cos")
                    nc.vector.tensor_mul(cosm[:mw], g_ps[:mw], rnb[:mw])
                    mx = smalls.tile([128, 8], F32, tag="mx")
                    nc.vector.max(out=mx[:mw], in_=cosm[:mw])
                    idxu = smalls.tile([128, 8], U32, tag="idxu")
                    nc.vector.max_index(out=idxu[:mw], in_max=mx[:mw],
                                        in_values=cosm[:mw])
                    ixf = idxpool.tile([128, 1], F32, tag="ixf")
                    nc.vector.tensor_copy(out=ixf[:mw], in_=idxu[:mw, 0:1])
                    idxf[b].append(ixf)

            def mlp_pair(p):
                """MLP for samples 2p, 2p+1 -> pred into csb[..., HW:]."""
                b0 = 2 * p
                pred_ps = [ps_pred.tile([128, NPAIR], F32, tag="pred",
                                        name=f"pred_ps_{p}_{m2}")
                           for m2 in range(KD)]
                for k in range(KH):
                    h_ps = ps_h.tile([128, NPAIR], F32, tag="h")
                    for kc in range(KD):
                        nc.tensor.matmul(
                            h_ps, w1sb[:, kc, k * 128:(k + 1) * 128],
                            xsb[:, kc, b0:b0 + 2, :],
                            start=(kc == 0), stop=(kc == KD - 1))
                    h_sb = hpool.tile([128, NPAIR], BF16, tag="h_sb")
                    nc.scalar.activation(out=h_sb, in_=h_ps, func=AF.Relu,
                                         bias=b1sb[:, k:k + 1], scale=1.0)
                    for m2 in range(KD):
                        nc.tensor.matmul(
                            pred_ps[m2],
                            w2sb[:, k, m2 * 128:(m2 + 1) * 128],
                            h_sb,
                            start=(k == 0), stop=(k == KH - 1))
                for m2 in range(KD):
                    nc.scalar.activation(
                        out=csb[:, m2, b0:b0 + 2, HW:],
                        in_=pred_ps[m2].rearrange("p (b n) -> p b n", n=HW),
                        func=AF.Identity, bias=b2sb[:, m2:m2 + 1], scale=1.0)

            def stage_c(b):
                """P-gram, selects, final cosine -> res[:, m*BSH+b]."""
                # |dt_j|^2 (squares on gpsimd to offload the vector engine)
                dtq = smalls.tile([128, KD, HW], BF16, tag="dtq")
                for k in range(KD):
                    nc.gpsimd.tensor_mul(dtq[:, k], csb[:, k, b, :HW],
                                         csb[:, k, b, :HW])
                dtn_ps = ps_small.tile([1, HW], F32, tag="small")
                for k in range(KD):
                    nc.tensor.matmul(dtn_ps, ones_b, dtq[:, k],
                                     start=(k == 0), stop=(k == KD - 1))
                dtn = smalls.tile([1, HW], F32, tag="dtn")
                nc.vector.tensor_copy(out=dtn, in_=dtn_ps)
                dtnb = smalls.tile([128, HW], F32, tag="dtnb")
                nc.gpsimd.partition_broadcast(dtnb, dtn)

                # per-m-tile selects; columns mi of (128,2) combine tiles.
                # dsel unwritten rows must be exact 0 (they land in res);
                # pden/dden unwritten rows must be >0 so sqrt/recip stay
                # finite (0 * NaN would poison res).
                dsel = smalls.tile([128, 2], F32, tag="dsel")
                nc.vector.memset(dsel, 0.0)
                pden = smalls.tile([128, 2], F32, tag="pden")
                nc.vector.memset(pden, 1.0)
                dden = smalls.tile([128, 2], F32, tag="dden")
                nc.vector.memset(dden, 1.0)
                for mi, (m0, mw) in enumerate(MT):
                    pg_ps = ps_big.tile([128, NPAIR], F32, tag="big")
                    for k in range(KD):
                        nc.tensor.matmul(
                            pg_ps[:mw],
                            csb[:, k, b, HW + m0: HW + m0 + mw],
                            csb[:, k, b, :],
                            start=(k == 0), stop=(k == KD - 1))
                    ixf = idxf[b][mi]
                    scr = cospool.tile([128, HW], F32, tag="scr")
                    nc.vector.scalar_tensor_tensor(
                        out=scr[:mw], in0=iota_j[:mw], scalar=ixf[:mw],
                        in1=pg_ps[:mw, :HW], op0=ALU.is_equal, op1=ALU.mult,
                        accum_out=dsel[:mw, mi:mi + 1])
                    scr2 = cospool.tile([128, HW], F32, tag="scr")
                    nc.vector.scalar_tensor_tensor(
                        out=scr2[:mw], in0=iota_d[:mw], scalar=float(m0),
                        in1=pg_ps[:mw, HW:], op0=ALU.is_equal, op1=ALU.mult,
                        accum_out=pden[:mw, mi:mi + 1])
                    scr3 = cospool.tile([128, HW], F32, tag="scr")
                    nc.vector.scalar_tensor_tensor(
                        out=scr3[:mw], in0=iota_j[:mw], scalar=ixf[:mw],
                        in1=dtnb[:mw], op0=ALU.is_equal, op1=ALU.mult,
                        accum_out=dden[:mw, mi:mi + 1])
                # cos = dsel * rsqrt(pden * dden), both m-tiles at once;
                # written straight into res[:, (mi, b)] via a strided view
                den = smalls.tile([128, 2], F32, tag="den")
                nc.vector.tensor_mul(den, pden, dden)
                nc.scalar.sqrt(out=den, in_=den)
                nc.vector.reciprocal_approx_fast(out=den, in_=den)
                res_mb = res.rearrange("p (m b) -> p m b", b=BSH)[:, :, b]
                nc.vector.tensor_mul(res_mb, den, dsel)

            # ---- schedule: MLP(p) leads its pair so PE has work during loads
            for b in range(BSH):
                if b % 2 == 0:
                    mlp_pair(b // 2)
                stage_a(b)
                if b % 2 == 1:
                    stage_c(b - 1)
                    stage_c(b)
                if b + 2 < BSH:
                    load_feats(b + 2)

            # ---- final partition reduction -> scalar partial sum
            sum_ps = ps_small.tile([1, 2 * BSH], F32, tag="small")
            nc.tensor.matmul(sum_ps, ones_f, res, start=True, stop=True)
            total = smalls.tile([1, 1], F32, tag="total")
            nc.vector.reduce_sum(out=total, in_=sum_ps,
                                 axis=mybir.AxisListType.X)
            nc.sync.dma_start(out=out.ap(), in_=total)

    nc.compile()
    return nc


_NC_CACHE = None


def _get_nc():
    global _NC_CACHE
    if _NC_CACHE is None:
        _NC_CACHE = build_nc()
    return _NC_CACHE


def make_in_maps(feat_on, feat_targ, dense_on, dense_targ, W1, b1, W2, b2):
    bf = ml_dtypes.bfloat16
    # feats: (64, 2048, 14, 14) -> (64, 128, 16, 196) partition-major bf16
    def feat_prep(a):
        a = np.asarray(a, np.float32).reshape(B_FULL, KF, 128, HW)
        return np.ascontiguousarray(a.transpose(0, 2, 1, 3)).astype(bf)

    # dense: (64, 256, 14, 14) -> (128, 2, 64, 196) bf16
    def dense_prep(a):
        a = np.asarray(a, np.float32).reshape(B_FULL, KD, 128, HW)
        return np.ascontiguousarray(a.transpose(2, 1, 0, 3)).astype(bf)

    f_on = feat_prep(feat_on)
    f_tg = feat_prep(feat_targ)
    d_on = dense_prep(dense_on)
    d_tg = dense_prep(dense_targ)
    # W1 (2048,256): lhsT layout [c_part, kd, hid] = W1[h, kd*128+p]
    w1t = np.ascontiguousarray(
        np.asarray(W1, np.float32).T.reshape(KD, 128, HID).transpose(1, 0, 2)
    ).astype(bf)
    # W2 (256,2048): lhsT layout [h_part, kh, cd] = W2[c, kh*128+p]
    w2t = np.ascontiguousarray(
        np.asarray(W2, np.float32).T.reshape(KH, 128, CD).transpose(1, 0, 2)
    ).astype(bf)
    b1r = np.ascontiguousarray(np.asarray(b1, np.float32).reshape(KH, 128).T)
    b2r = np.ascontiguousarray(np.asarray(b2, np.float32).reshape(KD, 128).T)
    in_maps = []
    for c in range(N_CORES):
        s = slice(c * BSH, (c + 1) * BSH)
        in_maps.append({
            "f_on": f_on[s], "f_tg": f_tg[s],
            "d_on": np.ascontiguousarray(d_on[:, :, s]),
            "d_tg": np.ascontiguousarray(d_tg[:, :, s]),
            "w1t": w1t, "w2t": w2t, "b1r": b1r, "b2r": b2r,
        })
    return in_maps


def finish(partials):
    S = float(np.sum(np.asarray(partials, np.float64)))
    return np.float32(-2.0 * S / (B_FULL * H * W) + 2.0)


def kernel(**inputs):
    from concourse.bass_utils import run_bass_kernel_spmd
    nc = _get_nc()
    in_maps = make_in_maps(**inputs)
    r = run_bass_kernel_spmd(nc, in_maps, core_ids=list(range(N_CORES)))
    partials = [r.results[c]["out"][0, 0] for c in range(N_CORES)]
    return np.asarray(finish(partials))


# revision 11
# speedup vs baseline: 1.0689x; 1.0689x over previous
"""DenseCL head loss kernel for Trainium2 (8 NeuronCores, batch-parallel).

Per-core shard: 8 of the 64 samples. On-device per sample:
  pred = W2 @ relu(W1 @ dense_on + b1) + b2            (MLP over channels)
  G    = feat_on^T @ feat_targ  (per-position gram)     -> argmax_j G[:,j]/|ft_j|
  P    = pred^T @ [dense_targ | pred]                   (dot + pred-norm diag)
  cos  = P[i, idx_i] / sqrt(|pred_i|^2 * |dt_idx_i|^2)
Core output = sum_i cos (scalar partial). Host combines partials:
  loss = -2 * S / (b*h*w) + 2

All matmuls run in bf16 with fp32 PSUM accumulation; the scalar tail
(norms, argmax compare, final cosine) is fp32. Inputs are cast to bf16 and
laid out in SBUF order (partition-major) on the host, so every device DMA
is a large fully-contiguous transfer.
"""

import numpy as np
import ml_dtypes

import concourse.bacc as bacc
import concourse.bass as bass
import concourse.mybir as mybir
import concourse.tile as tile

F32 = mybir.dt.float32
BF16 = mybir.dt.bfloat16
U32 = mybir.dt.uint32
AF = mybir.ActivationFunctionType
ALU = mybir.AluOpType

# problem shapes (hardcoded per spec)
B_FULL, CF, H, W = 64, 2048, 14, 14
CD, HID = 256, 2048
HW = H * W                       # 196
N_CORES = 8
BSH = B_FULL // N_CORES          # 8 samples per core
KF = CF // 128                   # 16 feat k-tiles
KD = CD // 128                   # 2 dense k-tiles
KH = HID // 128                  # 16 hidden k-tiles
MT = [(0, 128), (128, HW - 128)]  # m-tiles over the 196 positions
NPAIR = 2 * HW                   # 392: two samples side by side
FHALF = KF // 2                  # feat DMA split for pipelining


def build_nc():
    nc = bacc.Bacc("TRN2", target_bir_lowering=False, debug=False,
                   num_devices=N_CORES)

    # host pre-arranged, bf16, partition-major
    f_on = nc.dram_tensor("f_on", [BSH, 128, KF, HW], BF16, kind="ExternalInput")
    f_tg = nc.dram_tensor("f_tg", [BSH, 128, KF, HW], BF16, kind="ExternalInput")
    d_on = nc.dram_tensor("d_on", [128, KD, BSH, HW], BF16, kind="ExternalInput")
    d_tg = nc.dram_tensor("d_tg", [128, KD, BSH, HW], BF16, kind="ExternalInput")
    w1t = nc.dram_tensor("w1t", [128, KD, HID], BF16, kind="ExternalInput")
    w2t = nc.dram_tensor("w2t", [128, KH, CD], BF16, kind="ExternalInput")
    b1r = nc.dram_tensor("b1r", [128, KH], F32, kind="ExternalInput")
    b2r = nc.dram_tensor("b2r", [128, KD], F32, kind="ExternalInput")
    out = nc.dram_tensor("out", [1, 1], F32, kind="ExternalOutput")

    with tile.TileContext(nc) as tc:
        with (
            tc.tile_pool(name="singles", bufs=1) as singles,
            tc.tile_pool(name="fpool", bufs=4) as fpool,
            tc.tile_pool(name="sqpool", bufs=2) as sqpool,
            tc.tile_pool(name="hpool", bufs=3) as hpool,
            tc.tile_pool(name="cospool", bufs=3) as cospool,
            tc.tile_pool(name="smalls", bufs=3) as smalls,
            tc.tile_pool(name="idxpool", bufs=6) as idxpool,
            tc.tile_pool(name="ps_h", bufs=2, space="PSUM") as ps_h,
            tc.tile_pool(name="ps_pred", bufs=2, space="PSUM") as ps_pred,
            tc.tile_pool(name="ps_big", bufs=2, space="PSUM") as ps_big,
            tc.tile_pool(name="ps_small", bufs=2, space="PSUM") as ps_small,
        ):
            # ---- MLP inputs first: PE can start on the MLP while feats load
            w1sb = singles.tile([128, KD, HID], BF16)
            nc.sync.dma_start(out=w1sb, in_=w1t.ap())
            xsb = singles.tile([128, KD, BSH, HW], BF16)
            nc.sync.dma_start(out=xsb, in_=d_on.ap())
            w2sb = singles.tile([128, KH, CD], BF16)
            nc.sync.dma_start(out=w2sb, in_=w2t.ap())
            b1sb = singles.tile([128, KH], F32)
            nc.sync.dma_start(out=b1sb, in_=b1r.ap())
            b2sb = singles.tile([128, KD], F32)
            nc.sync.dma_start(out=b2sb, in_=b2r.ap())

            f1sb = {}
            f2sb = {}

            def load_feats(b):
                f1 = fpool.tile([128, KF * HW], BF16, tag="f1")
                f2 = fpool.tile([128, KF * HW], BF16, tag="f2")
                half = FHALF * HW
                for h0 in (0, 1):
                    nc.sync.dma_start(
                        out=f2[:, h0 * half:(h0 + 1) * half],
                        in_=f_tg.ap()[b, :, h0 * FHALF:(h0 + 1) * FHALF])
                    nc.sync.dma_start(
                        out=f1[:, h0 * half:(h0 + 1) * half],
                        in_=f_on.ap()[b, :, h0 * FHALF:(h0 + 1) * FHALF])
                f1sb[b] = f1
                f2sb[b] = f2

            for _b in range(4):
                load_feats(_b)

            # C holds [dense_targ | pred] per (k-tile, sample): width 392
            csb = singles.tile([128, KD, BSH, 2 * HW], BF16)
            for k in range(KD):
                nc.sync.dma_start(out=csb[:, k, :, :HW], in_=d_tg.ap()[:, k])

            ones_b = singles.tile([128, 1], BF16)
            nc.vector.memset(ones_b, 1.0)
            ones_f = singles.tile([128, 1], F32)
            nc.vector.memset(ones_f, 1.0)
            iota_j = singles.tile([128, HW], F32)
            nc.gpsimd.iota(iota_j, [[1, HW]], channel_multiplier=0,
                           allow_small_or_imprecise_dtypes=True)
            iota_d = singles.tile([128, HW], F32)  # value = n - p
            nc.gpsimd.iota(iota_d, [[1, HW]], channel_multiplier=-1,
                           allow_small_or_imprecise_dtypes=True)
            # result accumulator: res[p, m*BSH + b] = cos for position m*128+p
            res = singles.tile([128, 2 * BSH], F32)
            nc.vector.memset(res, 0.0)

            idxf = {}

            rnbs = {}

            def prenorm(b):
                """1/|ft_j| chain for sample b -> rnbs[b] (128,HW)."""
                f2 = f2sb[b]
                f2sq = sqpool.tile([128, KF * HW], BF16, tag="f2sq")
                half = FHALF * HW
                for h0 in (0, 1):
                    nc.vector.tensor_mul(
                        f2sq[:, h0 * half:(h0 + 1) * half],
                        f2[:, h0 * half:(h0 + 1) * half],
                        f2[:, h0 * half:(h0 + 1) * half])
                nrm_ps = ps_small.tile([1, HW], F32, tag="small")
                for k in range(KF):
                    nc.tensor.matmul(
                        nrm_ps, ones_b, f2sq[:, k * HW:(k + 1) * HW],
                        start=(k == 0), stop=(k == KF - 1))
                rn = smalls.tile([1, HW], F32, tag="rn")
                nc.scalar.sqrt(out=rn, in_=nrm_ps)
                nc.vector.reciprocal_approx_fast(out=rn, in_=rn)
                rnb = smalls.tile([128, HW], F32, tag="rnb", bufs=4)
                nc.gpsimd.partition_broadcast(rnb, rn)
                rnbs[b] = rnb

            def stage_a(b):
                """gram + argmax for sample b -> idxf[b] (per-mtile (mw,1))."""
                f1, f2 = f1sb[b], f2sb[b]
                rnb = rnbs[b]
                idxf[b] = []
                for mi, (m0, mw) in enumerate(MT):
                    g_ps = ps_big.tile([128, HW], F32, tag="big")
                    for k in range(KF):
                        nc.tensor.matmul(
                            g_ps[:mw],
                            f1[:, k * HW + m0: k * HW + m0 + mw],
                            f2[:, k * HW:(k + 1) * HW],
                            start=(k == 0), stop=(k == KF - 1))
                    cosm = cospool.tile([128, HW], F32, tag="cos")
                    nc.vector.tensor_mul(cosm[:mw], g_ps[:mw], rnb[:mw])
                    mx = smalls.tile([128, 8], F32, tag="mx")
                    nc.vector.max(out=mx[:mw], in_=cosm[:mw])
                    idxu = smalls.tile([128, 8], U32, tag="idxu")
                    nc.vector.max_index(out=idxu[:mw], in_max=mx[:mw],
                                        in_values=cosm[:mw])
                    ixf = idxpool.tile([128, 1], F32, tag="ixf")
                    nc.vector.tensor_copy(out=ixf[:mw], in_=idxu[:mw, 0:1])
                    idxf[b].append(ixf)

            def mlp_pair(p):
                """MLP for samples 2p, 2p+1 -> pred into csb[..., HW:]."""
                b0 = 2 * p
                pred_ps = [ps_pred.tile([128, NPAIR], F32, tag="pred",
                                        name=f"pred_ps_{p}_{m2}")
                           for m2 in range(KD)]
                for k in range(KH):
                    h_ps = ps_h.tile([128, NPAIR], F32, tag="h")
                    for kc in range(KD):
                        nc.tensor.matmul(
                            h_ps, w1sb[:, kc, k * 128:(k + 1) * 128],
                            xsb[:, kc, b0:b0 + 2, :],
                            start=(kc == 0), stop=(kc == KD - 1))
                    h_sb = hpool.tile([128, NPAIR], BF16, tag="h_sb")
                    nc.scalar.activation(out=h_sb, in_=h_ps, func=AF.Relu,
                                         bias=b1sb[:, k:k + 1], scale=1.0)
                    for m2 in range(KD):
                        nc.tensor.matmul(
                            pred_ps[m2],
                            w2sb[:, k, m2 * 128:(m2 + 1) * 128],
                            h_sb,
                            start=(k == 0), stop=(k == KH - 1))
                for m2 in range(KD):
                    nc.scalar.activation(
                        out=csb[:, m2, b0:b0 + 2, HW:],
                        in_=pred_ps[m2].rearrange("p (b n) -> p b n", n=HW),
                        func=AF.Identity, bias=b2sb[:, m2:m2 + 1], scale=1.0)

            # dense-target norms for ALL samples, computed upfront (they
            # only need the d_tg load): removes a 4-engine chain from the
            # steady-state critical path.
            dtnb_all = singles.tile([128, BSH, HW], F32)

            def dtn_block():
                for b in range(BSH):
                    dtq = smalls.tile([128, KD, HW], BF16, tag="dtq",
                                      name=f"dtq_{b}")
                    for k in range(KD):
                        nc.gpsimd.tensor_mul(dtq[:, k], csb[:, k, b, :HW],
                                             csb[:, k, b, :HW])
                    dtn_ps = ps_small.tile([1, HW], F32, tag="small",
                                           name=f"dtn_ps_{b}")
                    for k in range(KD):
                        nc.tensor.matmul(dtn_ps, ones_b, dtq[:, k],
                                         start=(k == 0), stop=(k == KD - 1))
                    dtn = smalls.tile([1, HW], F32, tag="dtn",
                                      name=f"dtn_{b}")
                    nc.vector.tensor_copy(out=dtn, in_=dtn_ps)
                    nc.gpsimd.partition_broadcast(dtnb_all[:, b], dtn)

            def stage_c(b):
                """P-gram, selects, final cosine -> res[:, m*BSH+b]."""
                dtnb = dtnb_all[:, b]
                # per-m-tile selects; columns mi of (128,2) combine tiles.
                # dsel unwritten rows must be exact 0 (they land in res);
                # pden/dden unwritten rows must be >0 so sqrt/recip stay
                # finite (0 * NaN would poison res).
                dsel = smalls.tile([128, 2], F32, tag="dsel")
                nc.vector.memset(dsel, 0.0)
                pden = smalls.tile([128, 2], F32, tag="pden")
                nc.vector.memset(pden, 1.0)
                dden = smalls.tile([128, 2], F32, tag="dden")
                nc.vector.memset(dden, 1.0)
                for mi, (m0, mw) in enumerate(MT):
                    pg_ps = ps_big.tile([128, NPAIR], F32, tag="big")
                    for k in range(KD):
                        nc.tensor.matmul(
                            pg_ps[:mw],
                            csb[:, k, b, HW + m0: HW + m0 + mw],
                            csb[:, k, b, :],
                            start=(k == 0), stop=(k == KD - 1))
                    ixf = idxf[b][mi]
                    scr = cospool.tile([128, HW], F32, tag="scr")
                    nc.vector.scalar_tensor_tensor(
                        out=scr[:mw], in0=iota_j[:mw], scalar=ixf[:mw],
                        in1=pg_ps[:mw, :HW], op0=ALU.is_equal, op1=ALU.mult,
                        accum_out=dsel[:mw, mi:mi + 1])
                    scr2 = cospool.tile([128, HW], F32, tag="scr")
                    nc.vector.scalar_tensor_tensor(
                        out=scr2[:mw], in0=iota_d[:mw], scalar=float(m0),
                        in1=pg_ps[:mw, HW:], op0=ALU.is_equal, op1=ALU.mult,
                        accum_out=pden[:mw, mi:mi + 1])
                    scr3 = cospool.tile([128, HW], F32, tag="scr")
                    nc.vector.scalar_tensor_tensor(
                        out=scr3[:mw], in0=iota_j[:mw], scalar=ixf[:mw],
                        in1=dtnb[:mw], op0=ALU.is_equal, op1=ALU.mult,
                        accum_out=dden[:mw, mi:mi + 1])
                # cos = dsel * rsqrt(pden * dden), both m-tiles at once;
                # written straight into res[:, (mi, b)] via a strided view
                den = smalls.tile([128, 2], F32, tag="den")
                nc.vector.tensor_mul(den, pden, dden)
                nc.scalar.sqrt(out=den, in_=den)
                nc.vector.reciprocal_approx_fast(out=den, in_=den)
                res_mb = res.rearrange("p (m b) -> p m b", b=BSH)[:, :, b]
                nc.vector.tensor_mul(res_mb, den, dsel)

            # ---- schedule: MLP(p) leads each pair (its inputs land first);
            # prenorm chains run on DVE/GPS while the PE does MLP+grams;
            # selects for this pair come after the next pair's DVE work is
            # already queued via the following iteration's prenorms.
            for p in range(BSH // 2):
                b0, b1 = 2 * p, 2 * p + 1
                mlp_pair(p)
                prenorm(b0)
                prenorm(b1)
                if p == 0:
                    dtn_block()
                stage_a(b0)
                stage_a(b1)
                stage_c(b0)
                stage_c(b1)
                if b0 + 4 < BSH:
                    load_feats(b0 + 4)
                    load_feats(b1 + 4)

            # ---- final partition reduction -> scalar partial sum
            sum_ps = ps_small.tile([1, 2 * BSH], F32, tag="small")
            nc.tensor.matmul(sum_ps, ones_f, res, start=True, stop=True)
            total = smalls.tile([1, 1], F32, tag="total")
            nc.vector.reduce_sum(out=total, in_=sum_ps,
                                 axis=mybir.AxisListType.X)
            nc.sync.dma_start(out=out.ap(), in_=total)

    nc.compile()
    return nc


_NC_CACHE = None


def _get_nc():
    global _NC_CACHE
    if _NC_CACHE is None:
        _NC_CACHE = build_nc()
    return _NC_CACHE


def make_in_maps(feat_on, feat_targ, dense_on, dense_targ, W1, b1, W2, b2):
    bf = ml_dtypes.bfloat16
    # feats: (64, 2048, 14, 14) -> (64, 128, 16, 196) partition-major bf16
    def feat_prep(a):
        a = np.asarray(a, np.float32).reshape(B_FULL, KF, 128, HW)
        return np.ascontiguousarray(a.transpose(0, 2, 1, 3)).astype(bf)

    # dense: (64, 256, 14, 14) -> (128, 2, 64, 196) bf16
    def dense_prep(a):
        a = np.asarray(a, np.float32).reshape(B_FULL, KD, 128, HW)
        return np.ascontiguousarray(a.transpose(2, 1, 0, 3)).astype(bf)

    f_on = feat_prep(feat_on)
    f_tg = feat_prep(feat_targ)
    d_on = dense_prep(dense_on)
    d_tg = dense_prep(dense_targ)
    # W1 (2048,256): lhsT layout [c_part, kd, hid] = W1[h, kd*128+p]
    w1t = np.ascontiguousarray(
        np.asarray(W1, np.float32).T.reshape(KD, 128, HID).transpose(1, 0, 2)
    ).astype(bf)
    # W2 (256,2048): lhsT layout [h_part, kh, cd] = W2[c, kh*128+p]
    w2t = np.ascontiguousarray(
        np.asarray(W2, np.float32).T.reshape(KH, 128, CD).transpose(1, 0, 2)
    ).astype(bf)
    b1r = np.ascontiguousarray(np.asarray(b1, np.float32).reshape(KH, 128).T)
    b2r = np.ascontiguousarray(np.asarray(b2, np.float32).reshape(KD, 128).T)
    in_maps = []
    for c in range(N_CORES):
        s = slice(c * BSH, (c + 1) * BSH)
        in_maps.append({
            "f_on": f_on[s], "f_tg": f_tg[s],
            "d_on": np.ascontiguousarray(d_on[:, :, s]),
            "d_tg": np.ascontiguousarray(d_tg[:, :, s]),
            "w1t": w1t, "w2t": w2t, "b1r": b1r, "b2r": b2r,
        })
    return in_maps


def finish(partials):
    S = float(np.sum(np.asarray(partials, np.float64)))
    return np.float32(-2.0 * S / (B_FULL * H * W) + 2.0)


def kernel(**inputs):
    from concourse.bass_utils import run_bass_kernel_spmd
    nc = _get_nc()
    in_maps = make_in_maps(**inputs)
    r = run_bass_kernel_spmd(nc, in_maps, core_ids=list(range(N_CORES)))
    partials = [r.results[c]["out"][0, 0] for c in range(N_CORES)]
    return np.asarray(finish(partials))


# revision 12
# speedup vs baseline: 1.0773x; 1.0078x over previous
"""DenseCL head loss kernel for Trainium2 (8 NeuronCores, batch-parallel).

Per-core shard: 8 of the 64 samples. On-device per sample:
  pred = W2 @ relu(W1 @ dense_on + b1) + b2            (MLP over channels)
  G    = feat_on^T @ feat_targ  (per-position gram)     -> argmax_j G[:,j]/|ft_j|
  P    = pred^T @ [dense_targ | pred]                   (dot + pred-norm diag)
  cos  = P[i, idx_i] / sqrt(|pred_i|^2 * |dt_idx_i|^2)
Core output = sum_i cos (scalar partial). Host combines partials:
  loss = -2 * S / (b*h*w) + 2

All matmuls run in bf16 with fp32 PSUM accumulation; the scalar tail
(norms, argmax compare, final cosine) is fp32. Inputs are cast to bf16 and
laid out in SBUF order (partition-major) on the host, so every device DMA
is a large fully-contiguous transfer.
"""

import numpy as np
import ml_dtypes

import concourse.bacc as bacc
import concourse.bass as bass
import concourse.mybir as mybir
import concourse.tile as tile

F32 = mybir.dt.float32
BF16 = mybir.dt.bfloat16
U32 = mybir.dt.uint32
AF = mybir.ActivationFunctionType
ALU = mybir.AluOpType

# problem shapes (hardcoded per spec)
B_FULL, CF, H, W = 64, 2048, 14, 14
CD, HID = 256, 2048
HW = H * W                       # 196
N_CORES = 8
BSH = B_FULL // N_CORES          # 8 samples per core
KF = CF // 128                   # 16 feat k-tiles
KD = CD // 128                   # 2 dense k-tiles
KH = HID // 128                  # 16 hidden k-tiles
MT = [(0, 128), (128, HW - 128)]  # m-tiles over the 196 positions
NPAIR = 2 * HW                   # 392: two samples side by side
FHALF = KF // 2                  # feat DMA split for pipelining


def build_nc():
    nc = bacc.Bacc("TRN2", target_bir_lowering=False, debug=False,
                   num_devices=N_CORES)

    # host pre-arranged, bf16, partition-major
    f_on = nc.dram_tensor("f_on", [BSH, 128, KF, HW], BF16, kind="ExternalInput")
    f_tg = nc.dram_tensor("f_tg", [BSH, 128, KF, HW], BF16, kind="ExternalInput")
    d_on = nc.dram_tensor("d_on", [128, KD, BSH, HW], BF16, kind="ExternalInput")
    d_tg = nc.dram_tensor("d_tg", [128, KD, BSH, HW], BF16, kind="ExternalInput")
    w1t = nc.dram_tensor("w1t", [128, KD, HID], BF16, kind="ExternalInput")
    w2t = nc.dram_tensor("w2t", [128, KH, CD], BF16, kind="ExternalInput")
    b1r = nc.dram_tensor("b1r", [128, KH], F32, kind="ExternalInput")
    b2r = nc.dram_tensor("b2r", [128, KD], F32, kind="ExternalInput")
    out = nc.dram_tensor("out", [1, 1], F32, kind="ExternalOutput")

    with tile.TileContext(nc) as tc:
        with (
            tc.tile_pool(name="singles", bufs=1) as singles,
            tc.tile_pool(name="fpool", bufs=4) as fpool,
            tc.tile_pool(name="sqpool", bufs=2) as sqpool,
            tc.tile_pool(name="hpool", bufs=3) as hpool,
            tc.tile_pool(name="cospool", bufs=3) as cospool,
            tc.tile_pool(name="smalls", bufs=3) as smalls,
            tc.tile_pool(name="idxpool", bufs=6) as idxpool,
            tc.tile_pool(name="ps_h", bufs=2, space="PSUM") as ps_h,
            tc.tile_pool(name="ps_pred", bufs=2, space="PSUM") as ps_pred,
            tc.tile_pool(name="ps_big", bufs=2, space="PSUM") as ps_big,
            tc.tile_pool(name="ps_small", bufs=2, space="PSUM") as ps_small,
        ):
            # ---- MLP inputs first: PE can start on the MLP while feats load
            w1sb = singles.tile([128, KD, HID], BF16)
            nc.sync.dma_start(out=w1sb, in_=w1t.ap())
            xsb = singles.tile([128, KD, BSH, HW], BF16)
            nc.sync.dma_start(out=xsb, in_=d_on.ap())
            w2sb = singles.tile([128, KH, CD], BF16)
            nc.sync.dma_start(out=w2sb, in_=w2t.ap())
            b1sb = singles.tile([128, KH], F32)
            nc.sync.dma_start(out=b1sb, in_=b1r.ap())
            b2sb = singles.tile([128, KD], F32)
            nc.sync.dma_start(out=b2sb, in_=b2r.ap())

            f1sb = {}
            f2sb = {}

            def load_feats(b):
                f1 = fpool.tile([128, KF * HW], BF16, tag="f1")
                f2 = fpool.tile([128, KF * HW], BF16, tag="f2")
                half = FHALF * HW
                for h0 in (0, 1):
                    nc.sync.dma_start(
                        out=f2[:, h0 * half:(h0 + 1) * half],
                        in_=f_tg.ap()[b, :, h0 * FHALF:(h0 + 1) * FHALF])
                    nc.sync.dma_start(
                        out=f1[:, h0 * half:(h0 + 1) * half],
                        in_=f_on.ap()[b, :, h0 * FHALF:(h0 + 1) * FHALF])
                f1sb[b] = f1
                f2sb[b] = f2

            for _b in range(4):
                load_feats(_b)

            # C holds [dense_targ | pred] per (k-tile, sample): width 392
            csb = singles.tile([128, KD, BSH, 2 * HW], BF16)
            for k in range(KD):
                nc.sync.dma_start(out=csb[:, k, :, :HW], in_=d_tg.ap()[:, k])

            ones_b = singles.tile([128, 1], BF16)
            nc.vector.memset(ones_b, 1.0)
            ones_f = singles.tile([128, 1], F32)
            nc.vector.memset(ones_f, 1.0)
            iota_j = singles.tile([128, HW], F32)
            nc.gpsimd.iota(iota_j, [[1, HW]], channel_multiplier=0,
                           allow_small_or_imprecise_dtypes=True)
            iota_d = singles.tile([128, HW], F32)  # value = n - p
            nc.gpsimd.iota(iota_d, [[1, HW]], channel_multiplier=-1,
                           allow_small_or_imprecise_dtypes=True)
            # result accumulator: res[p, m*BSH + b] = cos for position m*128+p
            res = singles.tile([128, 2 * BSH], F32)
            nc.vector.memset(res, 0.0)

            idxf = {}

            rnbs = {}

            def prenorm(b):
                """1/|ft_j| chain for sample b -> rnbs[b] (128,HW)."""
                f2 = f2sb[b]
                f2sq = sqpool.tile([128, KF * HW], BF16, tag="f2sq")
                half = FHALF * HW
                for h0 in (0, 1):
                    nc.vector.tensor_mul(
                        f2sq[:, h0 * half:(h0 + 1) * half],
                        f2[:, h0 * half:(h0 + 1) * half],
                        f2[:, h0 * half:(h0 + 1) * half])
                nrm_ps = ps_small.tile([1, HW], F32, tag="small")
                for k in range(KF):
                    nc.tensor.matmul(
                        nrm_ps, ones_b, f2sq[:, k * HW:(k + 1) * HW],
                        start=(k == 0), stop=(k == KF - 1))
                rn = smalls.tile([1, HW], F32, tag="rn")
                nc.scalar.sqrt(out=rn, in_=nrm_ps)
                nc.vector.reciprocal_approx_fast(out=rn, in_=rn)
                rnb = smalls.tile([128, HW], F32, tag="rnb", bufs=4)
                nc.gpsimd.partition_broadcast(rnb, rn)
                rnbs[b] = rnb

            def stage_a(b):
                """gram + argmax for sample b -> idxf[b] (per-mtile (mw,1))."""
                f1, f2 = f1sb[b], f2sb[b]
                rnb = rnbs[b]
                idxf[b] = []
                for mi, (m0, mw) in enumerate(MT):
                    g_ps = ps_big.tile([128, HW], F32, tag="big")
                    for k in range(KF):
                        nc.tensor.matmul(
                            g_ps[:mw],
                            f1[:, k * HW + m0: k * HW + m0 + mw],
                            f2[:, k * HW:(k + 1) * HW],
                            start=(k == 0), stop=(k == KF - 1))
                    cosm = cospool.tile([128, HW], F32, tag="cos")
                    nc.vector.tensor_mul(cosm[:mw], g_ps[:mw], rnb[:mw])
                    mx = smalls.tile([128, 8], F32, tag="mx")
                    nc.vector.max(out=mx[:mw], in_=cosm[:mw])
                    idxu = smalls.tile([128, 8], U32, tag="idxu")
                    nc.vector.max_index(out=idxu[:mw], in_max=mx[:mw],
                                        in_values=cosm[:mw])
                    ixf = idxpool.tile([128, 1], F32, tag="ixf")
                    nc.vector.tensor_copy(out=ixf[:mw], in_=idxu[:mw, 0:1])
                    idxf[b].append(ixf)

            def mlp_pair(p):
                """MLP for samples 2p, 2p+1 -> pred into csb[..., HW:]."""
                b0 = 2 * p
                pred_ps = [ps_pred.tile([128, NPAIR], F32, tag="pred",
                                        name=f"pred_ps_{p}_{m2}")
                           for m2 in range(KD)]
                for k in range(KH):
                    h_ps = ps_h.tile([128, NPAIR], F32, tag="h")
                    for kc in range(KD):
                        nc.tensor.matmul(
                            h_ps, w1sb[:, kc, k * 128:(k + 1) * 128],
                            xsb[:, kc, b0:b0 + 2, :],
                            start=(kc == 0), stop=(kc == KD - 1))
                    h_sb = hpool.tile([128, NPAIR], BF16, tag="h_sb")
                    nc.scalar.activation(out=h_sb, in_=h_ps, func=AF.Relu,
                                         bias=b1sb[:, k:k + 1], scale=1.0)
                    for m2 in range(KD):
                        nc.tensor.matmul(
                            pred_ps[m2],
                            w2sb[:, k, m2 * 128:(m2 + 1) * 128],
                            h_sb,
                            start=(k == 0), stop=(k == KH - 1))
                for m2 in range(KD):
                    nc.scalar.activation(
                        out=csb[:, m2, b0:b0 + 2, HW:],
                        in_=pred_ps[m2].rearrange("p (b n) -> p b n", n=HW),
                        func=AF.Identity, bias=b2sb[:, m2:m2 + 1], scale=1.0)

            # dense-target norms for ALL samples, computed upfront (they
            # only need the d_tg load): removes a 4-engine chain from the
            # steady-state critical path.
            dtnb_all = singles.tile([128, BSH, HW], F32)

            def dtn_block():
                for b in range(BSH):
                    dtq = smalls.tile([128, KD, HW], BF16, tag="dtq",
                                      name=f"dtq_{b}")
                    for k in range(KD):
                        nc.gpsimd.tensor_mul(dtq[:, k], csb[:, k, b, :HW],
                                             csb[:, k, b, :HW])
                    dtn_ps = ps_small.tile([1, HW], F32, tag="small",
                                           name=f"dtn_ps_{b}")
                    for k in range(KD):
                        nc.tensor.matmul(dtn_ps, ones_b, dtq[:, k],
                                         start=(k == 0), stop=(k == KD - 1))
                    dtn = smalls.tile([1, HW], F32, tag="dtn",
                                      name=f"dtn_{b}")
                    nc.vector.tensor_copy(out=dtn, in_=dtn_ps)
                    nc.gpsimd.partition_broadcast(dtnb_all[:, b], dtn)

            def stage_c(b):
                """P-gram, selects, final cosine -> res[:, m*BSH+b]."""
                dtnb = dtnb_all[:, b]
                # per-m-tile selects; columns mi of (128,2) combine tiles.
                # dsel unwritten rows must be exact 0 (they land in res);
                # pden/dden unwritten rows must be >0 so sqrt/recip stay
                # finite (0 * NaN would poison res).
                dsel = smalls.tile([128, 2], F32, tag="dsel")
                nc.vector.memset(dsel, 0.0)
                pden = smalls.tile([128, 2], F32, tag="pden")
                nc.vector.memset(pden, 1.0)
                dden = smalls.tile([128, 2], F32, tag="dden")
                nc.vector.memset(dden, 1.0)
                for mi, (m0, mw) in enumerate(MT):
                    pg_ps = ps_big.tile([128, NPAIR], F32, tag="big")
                    for k in range(KD):
                        nc.tensor.matmul(
                            pg_ps[:mw],
                            csb[:, k, b, HW + m0: HW + m0 + mw],
                            csb[:, k, b, :],
                            start=(k == 0), stop=(k == KD - 1))
                    ixf = idxf[b][mi]
                    scr = cospool.tile([128, HW], F32, tag="scr")
                    nc.vector.scalar_tensor_tensor(
                        out=scr[:mw], in0=iota_j[:mw], scalar=ixf[:mw],
                        in1=pg_ps[:mw, :HW], op0=ALU.is_equal, op1=ALU.mult,
                        accum_out=dsel[:mw, mi:mi + 1])
                    scr2 = cospool.tile([128, HW], F32, tag="scr")
                    nc.vector.scalar_tensor_tensor(
                        out=scr2[:mw], in0=iota_d[:mw], scalar=float(m0),
                        in1=pg_ps[:mw, HW:], op0=ALU.is_equal, op1=ALU.mult,
                        accum_out=pden[:mw, mi:mi + 1])
                    scr3 = cospool.tile([128, HW], F32, tag="scr")
                    nc.vector.scalar_tensor_tensor(
                        out=scr3[:mw], in0=iota_j[:mw], scalar=ixf[:mw],
                        in1=dtnb[:mw], op0=ALU.is_equal, op1=ALU.mult,
                        accum_out=dden[:mw, mi:mi + 1])
                # cos = dsel * rsqrt(pden * dden), both m-tiles at once;
                # written straight into res[:, (mi, b)] via a strided view
                den = smalls.tile([128, 2], F32, tag="den")
                nc.vector.tensor_mul(den, pden, dden)
                nc.scalar.sqrt(out=den, in_=den)
                nc.vector.reciprocal_approx_fast(out=den, in_=den)
                res_mb = res.rearrange("p (m b) -> p m b", b=BSH)[:, :, b]
                nc.vector.tensor_mul(res_mb, den, dsel)

            # ---- schedule: MLP(p) leads each pair (its inputs land first);
            # prenorm chains run on DVE/GPS while the PE does MLP+grams;
            # selects for this pair come after the next pair's DVE work is
            # already queued via the following iteration's prenorms.
            for p in range(BSH // 2):
                b0, b1 = 2 * p, 2 * p + 1
                with nc.named_scope(f"mlp_{p}"):
                    mlp_pair(p)
                with nc.named_scope(f"prenorm_{p}"):
                    prenorm(b0)
                    prenorm(b1)
                if p == 0:
                    with nc.named_scope("dtn_block"):
                        dtn_block()
                with nc.named_scope(f"gram_{b0}"):
                    stage_a(b0)
                with nc.named_scope(f"gram_{b1}"):
                    stage_a(b1)
                with nc.named_scope(f"selc_{p}"):
                    stage_c(b0)
                    stage_c(b1)
                if b0 + 4 < BSH:
                    with nc.named_scope(f"load_{p}"):
                        load_feats(b0 + 4)
                        load_feats(b1 + 4)

            # ---- final partition reduction -> scalar partial sum
            sum_ps = ps_small.tile([1, 2 * BSH], F32, tag="small")
            nc.tensor.matmul(sum_ps, ones_f, res, start=True, stop=True)
            total = smalls.tile([1, 1], F32, tag="total")
            nc.vector.reduce_sum(out=total, in_=sum_ps,
                                 axis=mybir.AxisListType.X)
            nc.sync.dma_start(out=out.ap(), in_=total)

    nc.compile()
    return nc


_NC_CACHE = None


def _get_nc():
    global _NC_CACHE
    if _NC_CACHE is None:
        _NC_CACHE = build_nc()
    return _NC_CACHE


def make_in_maps(feat_on, feat_targ, dense_on, dense_targ, W1, b1, W2, b2):
    bf = ml_dtypes.bfloat16
    # feats: (64, 2048, 14, 14) -> (64, 128, 16, 196) partition-major bf16
    def feat_prep(a):
        a = np.asarray(a, np.float32).reshape(B_FULL, KF, 128, HW)
        return np.ascontiguousarray(a.transpose(0, 2, 1, 3)).astype(bf)

    # dense: (64, 256, 14, 14) -> (128, 2, 64, 196) bf16
    def dense_prep(a):
        a = np.asarray(a, np.float32).reshape(B_FULL, KD, 128, HW)
        return np.ascontiguousarray(a.transpose(2, 1, 0, 3)).astype(bf)

    f_on = feat_prep(feat_on)
    f_tg = feat_prep(feat_targ)
    d_on = dense_prep(dense_on)
    d_tg = dense_prep(dense_targ)
    # W1 (2048,256): lhsT layout [c_part, kd, hid] = W1[h, kd*128+p]
    w1t = np.ascontiguousarray(
        np.asarray(W1, np.float32).T.reshape(KD, 128, HID).transpose(1, 0, 2)
    ).astype(bf)
    # W2 (256,2048): lhsT layout [h_part, kh, cd] = W2[c, kh*128+p]
    w2t = np.ascontiguousarray(
        np.asarray(W2, np.float32).T.reshape(KH, 128, CD).transpose(1, 0, 2)
    ).astype(bf)
    b1r = np.ascontiguousarray(np.asarray(b1, np.float32).reshape(KH, 128).T)
    b2r = np.ascontiguousarray(np.asarray(b2, np.float32).reshape(KD, 128).T)
    in_maps = []
    for c in range(N_CORES):
        s = slice(c * BSH, (c + 1) * BSH)
        in_maps.append({
            "f_on": f_on[s], "f_tg": f_tg[s],
            "d_on": np.ascontiguousarray(d_on[:, :, s]),
            "d_tg": np.ascontiguousarray(d_tg[:, :, s]),
            "w1t": w1t, "w2t": w2t, "b1r": b1r, "b2r": b2r,
        })
    return in_maps


def finish(partials):
    S = float(np.sum(np.asarray(partials, np.float64)))
    return np.float32(-2.0 * S / (B_FULL * H * W) + 2.0)


def kernel(**inputs):
    from concourse.bass_utils import run_bass_kernel_spmd
    nc = _get_nc()
    in_maps = make_in_maps(**inputs)
    r = run_bass_kernel_spmd(nc, in_maps, core_ids=list(range(N_CORES)))
    partials = [r.results[c]["out"][0, 0] for c in range(N_CORES)]
    return np.asarray(finish(partials))
